# revision 5
# baseline (speedup 1.0000x reference)
"""Multi-head attention block (B=2, N=2048, C=1024, H=16, D=64) on 8 TRN2
NeuronCores.

Sharding: tensor-parallel over heads — 2 heads per core, both batch elements.
Each core computes qkv for its 2 heads, full attention for its 4 (batch, head)
pairs, and a partial output projection over its 128 columns of the attention
output. The host sums the 8 partial projections and adds the bias.

Device-side layout (per core):
  - host feeds x transposed (xT [1024, 4096]) plus per-core transposed weight
    slices, so no activation transposes are needed on device for the linears.
  - qkvT [o, r] = wT_slice.T @ xT computed with o on partitions: q/k land
    d-major ([2*64, 4096]) ready to be S-matmul operands; v is PE-transposed
    into m-major V' tiles [128, 65] with an appended ones row, so the P@V
    matmul accumulates the softmax denominator for free.
  - S computed transposed (ST [keys, queries]) so exp(ST) is directly the
    moving operand of the P@V matmul — no P transposes.
  - softmax has no max-subtraction (logits are O(5) here; exp is safe in f32)
    and normalization is applied to the 64-row OT via a K=1 ones-matmul
    broadcast of 1/rowsum.

Matmul dtypes: float32r (tf32-like, ~1e-4 rel err) for qkv/S/proj; bf16 for
the P@V matmul (P in [0,1]; errors average out over 2048 keys).
"""
import sys

sys.path.insert(0, "/opt/trn_rl_repo")

import numpy as np

B = 2
N = 2048
C = 1024
H = 16
D = 64
R = B * N            # 4096 flattened rows
NCORES = 8
HPC = H // NCORES    # heads per core = 2
SCALE = 1.0 / np.sqrt(D)  # 0.125

_NC_CACHE = None


def build_nc():
    import concourse.bass as bass
    import concourse.tile as tile
    from concourse import bacc, mybir
    from concourse.masks import make_identity

    F32 = mybir.dt.float32
    F32R = mybir.dt.float32r
    BF16 = mybir.dt.bfloat16
    Exp = mybir.ActivationFunctionType.Exp

    nc = bacc.Bacc("TRN2", target_bir_lowering=False, debug=False,
                   num_devices=NCORES)

    xT_d = nc.declare_dram_parameter("xT", [C, R], F32R, isOutput=False)
    wqkvT_d = nc.declare_dram_parameter("wqkvT", [C, 3 * 2 * D], F32R,
                                        isOutput=False)
    wprojT_d = nc.declare_dram_parameter("wprojT", [2 * D, C], F32R,
                                         isOutput=False)
    ones_d = nc.declare_dram_parameter("ones64", [1, D], F32R, isOutput=False)
    y_d = nc.declare_dram_parameter("y", [R, C], F32, isOutput=True)

    O3 = 3 * 2 * D   # 384 qkv output rows per core
    CC = C // 128    # 8 contraction chunks
    NMC = N // 128   # 16 key chunks per (b, head)

    with tile.TileContext(nc) as tc:
        with (
            tc.tile_pool(name="const", bufs=1) as const,
            tc.tile_pool(name="qkvT", bufs=1) as qkvp,
            tc.tile_pool(name="vprime", bufs=1) as vpp,
            tc.tile_pool(name="otbuf", bufs=1) as otp,
        ):
            # ---- constants ----
            wqkv_sb = const.tile([128, CC, O3], F32R)
            nc.sync.dma_start(
                wqkv_sb[:], wqkvT_d.rearrange("(a p) o -> p a o", p=128))
            wproj_sb = const.tile([128, C], F32R)
            nc.sync.dma_start(wproj_sb[:], wprojT_d[:])
            ident = const.tile([128, 128], F32)
            make_identity(nc, ident[:])
            ones_col = const.tile([1, D], F32R)
            nc.sync.dma_start(ones_col[:], ones_d[:])

            # ---- persistent activations ----
            qT = qkvp.tile([128, R], F32R)   # rows: [q_h0 | q_h1] d-major
            kT = qkvp.tile([128, R], F32R)
            vT = qkvp.tile([128, R], F32)    # f32: feeds PE transposes only
            # V' per (b, hl): [128 keys-chunk, 16 chunks, 64+1]
            vprime = [[vpp.tile([128, NMC, D + 1], BF16, tag=f"vp{b}{hl}",
                                name=f"vp{b}{hl}")
                       for hl in range(HPC)] for b in range(B)]
            ot = otp.tile([128, R], F32R)    # normalized attention out, c-major

            # ================= phase 1: qkv projection =================
            with (
                tc.tile_pool(name="xt", bufs=3) as xtp,
                tc.tile_pool(name="qkps", bufs=2, space="PSUM") as qkps,
                tc.tile_pool(name="vtps", bufs=2, space="PSUM") as vtps,
            ):
                # ones rows of V' (bf16 1.0 exact); transposes fill [:, :, 0:D]
                for b in range(B):
                    for hl in range(HPC):
                        nc.gpsimd.memset(vprime[b][hl][:, :, D:D + 1], 1.0)

                for rb in range(R // 512):
                    col0 = rb * 512
                    xt = xtp.tile([128, CC, 512], F32R, tag="xt")
                    nc.sync.dma_start(
                        xt[:],
                        xT_d[:, col0:col0 + 512].rearrange(
                            "(a p) r -> p a r", p=128))
                    for ob in range(3):
                        dst = (qT, kT, vT)[ob]
                        ps = qkps.tile([128, 512], F32, tag="qk")
                        for cc in range(CC):
                            nc.tensor.matmul(
                                ps[:],
                                wqkv_sb[:, cc, ob * 128:(ob + 1) * 128],
                                xt[:, cc, :],
                                start=(cc == 0), stop=(cc == CC - 1),
                            )
                        nc.vector.tensor_copy(dst[:, col0:col0 + 512], ps[:])

                    # V' transposes for the v columns that just landed
                    for hl in range(HPC):
                        for i128 in range(4):
                            col = col0 + i128 * 128
                            b = col // N
                            mc = (col % N) // 128
                            pt = vtps.tile([128, D], F32, tag="vt")
                            nc.tensor.transpose(
                                pt[:],
                                vT[hl * D:(hl + 1) * D, col:col + 128],
                                ident[hl * D:(hl + 1) * D, hl * D:(hl + 1) * D],
                            )
                            nc.vector.tensor_copy(
                                vprime[b][hl][:, mc, 0:D], pt[:])

            # ================= phase 2: attention =================
            with (
                tc.tile_pool(name="stps", bufs=2, space="PSUM") as stps,
                tc.tile_pool(name="otps", bufs=1, space="PSUM") as otps,
                tc.tile_pool(name="et", bufs=3) as etp,
                tc.tile_pool(name="small", bufs=2) as small,
            ):
                for b in range(B):
                    for hl in range(HPC):
                        p0 = hl * D
                        rlo = b * N
                        otp_ps = otps.tile([D + 1, N], F32, tag="ot")
                        for mc in range(NMC):
                            kslice = kT[p0:p0 + D,
                                        rlo + mc * 128:rlo + (mc + 1) * 128]
                            st_tiles = []
                            for half in range(2):
                                st = stps.tile([128, 1024], F32, tag="st")
                                for j in range(2):
                                    qc = half * 1024 + j * 512
                                    nc.tensor.matmul(
                                        st[:, j * 512:(j + 1) * 512],
                                        kslice,
                                        qT[p0:p0 + D, rlo + qc:rlo + qc + 512],
                                        start=True, stop=True,
                                    )
                                st_tiles.append(st)
                            et = etp.tile([128, N], BF16, tag="et")
                            for half in range(2):
                                nc.scalar.activation(
                                    et[:, half * 1024:(half + 1) * 1024],
                                    st_tiles[half][:], Exp, scale=SCALE)
                            for qc in range(4):
                                nc.tensor.matmul(
                                    otp_ps[:, qc * 512:(qc + 1) * 512],
                                    vprime[b][hl][:, mc, :],
                                    et[:, qc * 512:(qc + 1) * 512],
                                    start=(mc == 0), stop=(mc == NMC - 1),
                                )
                        # softmax denominators -> broadcast -> normalize
                        rd = small.tile([1, N], F32, tag="rd")
                        nc.vector.reciprocal(rd[:], otp_ps[D:D + 1, :])
                        rdr = small.tile([1, N], F32R, tag="rdr")
                        nc.vector.tensor_copy(rdr[:], rd[:])
                        rbig = small.tile([D, N], F32, tag="rbig")
                        for half in range(2):
                            bp = stps.tile([D, 1024], F32, tag="st")
                            for j in range(2):
                                qc = half * 1024 + j * 512
                                nc.tensor.matmul(
                                    bp[:, j * 512:(j + 1) * 512],
                                    ones_col[:],
                                    rdr[:, qc:qc + 512],
                                    start=True, stop=True,
                                )
                            nc.vector.tensor_copy(
                                rbig[:, half * 1024:(half + 1) * 1024], bp[:])
                        nc.vector.tensor_mul(
                            ot[p0:p0 + D, rlo:rlo + N],
                            otp_ps[0:D, :], rbig[:])

            # ================= phase 3: output projection =================
            with (
                tc.tile_pool(name="yps", bufs=2, space="PSUM") as yps,
                tc.tile_pool(name="ysb", bufs=3) as ysbp,
            ):
                for rb in range(R // 128):
                    yp = yps.tile([128, C], F32, tag="y")
                    for j in range(2):
                        nc.tensor.matmul(
                            yp[:, j * 512:(j + 1) * 512],
                            ot[:, rb * 128:(rb + 1) * 128],
                            wproj_sb[:, j * 512:(j + 1) * 512],
                            start=True, stop=True,
                        )
                    ysb = ysbp.tile([128, C], F32, tag="ysb")
                    nc.vector.tensor_copy(ysb[:], yp[:])
                    nc.sync.dma_start(
                        y_d[rb * 128:(rb + 1) * 128, :], ysb[:])

    nc.compile()
    return nc


def get_nc():
    global _NC_CACHE
    if _NC_CACHE is None:
        _NC_CACHE = build_nc()
    return _NC_CACHE


def make_in_maps(x, w_qkv, w_proj):
    x = np.asarray(x, dtype=np.float32)
    w_qkv = np.asarray(w_qkv, dtype=np.float32)
    w_proj = np.asarray(w_proj, dtype=np.float32)
    xT = np.ascontiguousarray(x.reshape(R, C).T)
    ones = np.ones((1, D), dtype=np.float32)
    in_maps = []
    for i in range(NCORES):
        h0, h1 = HPC * i, HPC * i + 1
        rows = []
        for part in range(3):  # q, k, v
            for h in (h0, h1):
                lo = part * C + h * D
                rows.append(w_qkv[lo:lo + D])
        w_slice = np.concatenate(rows, axis=0)           # [384, 1024]
        wqkvT = np.ascontiguousarray(w_slice.T)          # [1024, 384]
        cols = np.r_[h0 * D:(h0 + 1) * D, h1 * D:(h1 + 1) * D]
        wprojT = np.ascontiguousarray(w_proj[:, cols].T)  # [128, 1024]
        in_maps.append({
            "xT": xT, "wqkvT": wqkvT, "wprojT": wprojT, "ones64": ones,
        })
    return in_maps


def kernel(x, w_qkv, w_proj, b_proj):
    from concourse.bass_utils import run_bass_kernel_spmd

    nc = get_nc()
    in_maps = make_in_maps(x, w_qkv, w_proj)
    res = run_bass_kernel_spmd(nc, in_maps, core_ids=list(range(NCORES)))
    y = np.zeros((R, C), dtype=np.float32)
    for r in res.results:
        y += r["y"]
    y += np.asarray(b_proj, dtype=np.float32)[None, :]
    return y.reshape(B, N, C)


# revision 6
# speedup vs baseline: 1.0566x; 1.0566x over previous
"""Multi-head attention block (B=2, N=2048, C=1024, H=16, D=64) on 8 TRN2
NeuronCores.

Sharding: tensor-parallel over heads — 2 heads per core, both batch elements.
Each core computes qkv for its 2 heads, full attention for its 4 (batch, head)
pairs, and a partial output projection over its 128 columns of the attention
output. The host sums the 8 partial projections and adds the bias.

Device-side layout (per core):
  - host feeds x transposed (xT [1024, 4096]) plus per-core transposed weight
    slices, so no activation transposes are needed on device for the linears.
  - qkvT [o, r] = wT_slice.T @ xT computed with o on partitions: q/k land
    d-major ([2*64, 4096]) ready to be S-matmul operands; v is PE-transposed
    into m-major V' tiles [128, 65] with an appended ones row, so the P@V
    matmul accumulates the softmax denominator for free.
  - S computed transposed (ST [keys, queries]) so exp(ST) is directly the
    moving operand of the P@V matmul — no P transposes.
  - softmax has no max-subtraction (logits are O(5) here; exp is safe in f32).
    Normalization runs off the critical path: unnormalized OT + denominator
    row are evicted to SBUF, then reciprocal (DVE) + partition_broadcast
    (GpSimd) + in-place multiply (DVE) overlap the next pair's matmuls.
  - proj for batch 0 is emitted between the two batches' attention so its
    PSUM use (borrowed from the ST tag), evictions, and output DMA overlap
    batch 1's attention.

Matmul dtypes: float32r (~1e-4 rel err) for qkv/S/proj; bf16 for the P@V
matmul (P in [0,1]; errors average out over 2048 keys).
"""
import sys

sys.path.insert(0, "/opt/trn_rl_repo")

import numpy as np

B = 2
N = 2048
C = 1024
H = 16
D = 64
R = B * N            # 4096 flattened rows
NCORES = 8
HPC = H // NCORES    # heads per core = 2
SCALE = 1.0 / np.sqrt(D)  # 0.125

_NC_CACHE = None


def build_nc():
    import concourse.bass as bass
    import concourse.tile as tile
    from concourse import bacc, mybir
    from concourse.masks import make_identity

    F32 = mybir.dt.float32
    F32R = mybir.dt.float32r
    BF16 = mybir.dt.bfloat16
    Exp = mybir.ActivationFunctionType.Exp

    nc = bacc.Bacc("TRN2", target_bir_lowering=False, debug=False,
                   num_devices=NCORES)

    xT_d = nc.declare_dram_parameter("xT", [C, R], F32R, isOutput=False)
    wqkvT_d = nc.declare_dram_parameter("wqkvT", [C, 3 * 2 * D], F32R,
                                        isOutput=False)
    wprojT_d = nc.declare_dram_parameter("wprojT", [2 * D, C], F32R,
                                         isOutput=False)
    y_d = nc.declare_dram_parameter("y", [R, C], F32, isOutput=True)

    O3 = 3 * 2 * D   # 384 qkv output rows per core
    CC = C // 128    # 8 contraction chunks
    NMC = N // 128   # 16 key chunks per (b, head)

    with tile.TileContext(nc) as tc:
        with (
            tc.tile_pool(name="const", bufs=1) as const,
            tc.tile_pool(name="qkvT", bufs=1) as qkvp,
            tc.tile_pool(name="vprime", bufs=1) as vpp,
            tc.tile_pool(name="otbuf", bufs=1) as otp,
        ):
            # ---- constants ----
            wqkv_sb = const.tile([128, CC, O3], F32R)
            nc.sync.dma_start(
                wqkv_sb[:], wqkvT_d.rearrange("(a p) o -> p a o", p=128))
            wproj_sb = const.tile([128, C], F32R)
            nc.sync.dma_start(wproj_sb[:], wprojT_d[:])
            ident = const.tile([128, 128], F32)
            make_identity(nc, ident[:])

            # ---- persistent activations ----
            qT = qkvp.tile([128, R], F32R)   # rows: [q_h0 | q_h1] d-major
            kT = qkvp.tile([128, R], F32R)
            vprime = [[vpp.tile([128, NMC, D + 1], BF16, tag=f"vp{b}{hl}",
                                name=f"vp{b}{hl}")
                       for hl in range(HPC)] for b in range(B)]
            ot = otp.tile([128, R], F32R)    # normalized attention out, c-major

            # ================= phase 1: qkv projection =================
            with (
                tc.tile_pool(name="vtbuf", bufs=1) as vtp,
                tc.tile_pool(name="xt", bufs=3) as xtp,
                tc.tile_pool(name="qkps", bufs=2, space="PSUM") as qkps,
                tc.tile_pool(name="vtps", bufs=2, space="PSUM") as vtps,
                nc.named_scope("qkv"),
            ):
                vT = vtp.tile([128, R], F32)
                # ones rows of V' (bf16 1.0 exact); transposes fill [:, :, 0:D]
                for b in range(B):
                    for hl in range(HPC):
                        nc.gpsimd.memset(vprime[b][hl][:, :, D:D + 1], 1.0)

                for rb in range(R // 512):
                    col0 = rb * 512
                    xt = xtp.tile([128, CC, 512], F32R, tag="xt")
                    nc.sync.dma_start(
                        xt[:],
                        xT_d[:, col0:col0 + 512].rearrange(
                            "(a p) r -> p a r", p=128))
                    for ob in range(3):
                        dst = (qT, kT, vT)[ob]
                        ps = qkps.tile([128, 512], F32, tag="qk")
                        for cc in range(CC):
                            nc.tensor.matmul(
                                ps[:],
                                wqkv_sb[:, cc, ob * 128:(ob + 1) * 128],
                                xt[:, cc, :],
                                start=(cc == 0), stop=(cc == CC - 1),
                            )
                        nc.vector.tensor_copy(dst[:, col0:col0 + 512], ps[:])

                    # V' transposes for the v columns that just landed
                    for hl in range(HPC):
                        for i128 in range(4):
                            col = col0 + i128 * 128
                            b = col // N
                            mc = (col % N) // 128
                            pt = vtps.tile([128, D], F32, tag="vt")
                            nc.tensor.transpose(
                                pt[:],
                                vT[hl * D:(hl + 1) * D, col:col + 128],
                                ident[hl * D:(hl + 1) * D,
                                      hl * D:(hl + 1) * D],
                            )
                            nc.vector.tensor_copy(
                                vprime[b][hl][:, mc, 0:D], pt[:])

            # ============ phase 2+3: attention / normalize / proj ==========
            with (
                tc.tile_pool(name="stps", bufs=2, space="PSUM") as stps,
                tc.tile_pool(name="otps", bufs=1, space="PSUM") as otps,
                tc.tile_pool(name="et", bufs=3) as etp,
                tc.tile_pool(name="small", bufs=2) as small,
                tc.tile_pool(name="ysb", bufs=4) as ysbp,
            ):
                def attention_pair(b, hl):
                    p0 = hl * D
                    rlo = b * N
                    otp_ps = otps.tile([D + 1, N], F32, tag="ot", name="otps")
                    for mc in range(NMC):
                        kslice = kT[p0:p0 + D,
                                    rlo + mc * 128:rlo + (mc + 1) * 128]
                        st_tiles = []
                        for half in range(2):
                            st = stps.tile([128, 1024], F32, tag="st",
                                           name="st")
                            for j in range(2):
                                qc = half * 1024 + j * 512
                                nc.tensor.matmul(
                                    st[:, j * 512:(j + 1) * 512],
                                    kslice,
                                    qT[p0:p0 + D, rlo + qc:rlo + qc + 512],
                                    start=True, stop=True,
                                )
                            st_tiles.append(st)
                        et = etp.tile([128, N], BF16, tag="et", name="et")
                        for half in range(2):
                            nc.scalar.activation(
                                et[:, half * 1024:(half + 1) * 1024],
                                st_tiles[half][:], Exp, scale=SCALE)
                        for qc in range(4):
                            nc.tensor.matmul(
                                otp_ps[:, qc * 512:(qc + 1) * 512],
                                vprime[b][hl][:, mc, :],
                                et[:, qc * 512:(qc + 1) * 512],
                                start=(mc == 0), stop=(mc == NMC - 1),
                            )
                    # single fast eviction releases the OT' psum; the
                    # normalize chain below runs off the critical path.
                    otu = small.tile([D + 1, N], F32, tag="otu", name="otu")
                    nc.vector.tensor_copy(otu[:], otp_ps[:])
                    rinv = small.tile([1, N], F32, tag="rinv", name="rinv")
                    nc.vector.reciprocal(rinv[:], otu[D:D + 1, :])
                    rbig = small.tile([D, N], F32, tag="rbig", name="rbig")
                    nc.gpsimd.partition_broadcast(rbig[:], rinv[:])
                    nc.vector.tensor_mul(
                        ot[p0:p0 + D, rlo:rlo + N], otu[0:D, :], rbig[:])

                def proj_b(b):
                    for rbi in range(N // 128):
                        rb = b * (N // 128) + rbi
                        yp = stps.tile([128, C], F32, tag="st", name="yp")
                        for j in range(2):
                            nc.tensor.matmul(
                                yp[:, j * 512:(j + 1) * 512],
                                ot[:, rb * 128:(rb + 1) * 128],
                                wproj_sb[:, j * 512:(j + 1) * 512],
                                start=True, stop=True,
                            )
                        ysb = ysbp.tile([128, C], F32, tag="ysb", name="ysb")
                        nc.vector.tensor_copy(ysb[:, 0:512], yp[:, 0:512])
                        nc.scalar.copy(ysb[:, 512:1024], yp[:, 512:1024])
                        nc.sync.dma_start(
                            y_d[rb * 128:(rb + 1) * 128, :], ysb[:])

                for b in range(B):
                    for hl in range(HPC):
                        with nc.named_scope(f"attn{b}{hl}"):
                            attention_pair(b, hl)
                    with nc.named_scope(f"proj{b}"):
                        proj_b(b)

    nc.compile()
    return nc


def get_nc():
    global _NC_CACHE
    if _NC_CACHE is None:
        _NC_CACHE = build_nc()
    return _NC_CACHE


def make_in_maps(x, w_qkv, w_proj):
    x = np.asarray(x, dtype=np.float32)
    w_qkv = np.asarray(w_qkv, dtype=np.float32)
    w_proj = np.asarray(w_proj, dtype=np.float32)
    xT = np.ascontiguousarray(x.reshape(R, C).T)
    in_maps = []
    for i in range(NCORES):
        h0, h1 = HPC * i, HPC * i + 1
        rows = []
        for part in range(3):  # q, k, v
            for h in (h0, h1):
                lo = part * C + h * D
                rows.append(w_qkv[lo:lo + D])
        w_slice = np.concatenate(rows, axis=0)           # [384, 1024]
        wqkvT = np.ascontiguousarray(w_slice.T)          # [1024, 384]
        cols = np.r_[h0 * D:(h0 + 1) * D, h1 * D:(h1 + 1) * D]
        wprojT = np.ascontiguousarray(w_proj[:, cols].T)  # [128, 1024]
        in_maps.append({"xT": xT, "wqkvT": wqkvT, "wprojT": wprojT})
    return in_maps


def kernel(x, w_qkv, w_proj, b_proj):
    from concourse.bass_utils import run_bass_kernel_spmd

    nc = get_nc()
    in_maps = make_in_maps(x, w_qkv, w_proj)
    res = run_bass_kernel_spmd(nc, in_maps, core_ids=list(range(NCORES)))
    y = np.zeros((R, C), dtype=np.float32)
    for r in res.results:
        y += r["y"]
    y += np.asarray(b_proj, dtype=np.float32)[None, :]
    return y.reshape(B, N, C)


# revision 8
# speedup vs baseline: 1.0980x; 1.0391x over previous
"""Multi-head attention block (B=2, N=2048, C=1024, H=16, D=64) on 8 TRN2
NeuronCores.

Sharding: tensor-parallel over heads — 2 heads per core, both batch elements.
Each core computes qkv for its 2 heads, full attention for its 4 (batch, head)
pairs, and a partial output projection over its 128 columns of the attention
output. The host sums the 8 partial projections and adds the bias.

Device-side layout (per core):
  - host feeds x transposed (xT [1024, 4096]) plus per-core transposed weight
    slices, so no activation transposes are needed on device for the linears.
  - qkvT [o, r] = wT_slice.T @ xT computed with o on partitions: q/k land
    d-major ([2*64, 4096]) ready to be S-matmul operands; v is PE-transposed
    into m-major V' tiles [128, 65] with an appended ones row, so the P@V
    matmul accumulates the softmax denominator for free.
  - S computed transposed (ST [keys, queries]) so exp(ST) is directly the
    moving operand of the P@V matmul — no P transposes.
  - softmax has no max-subtraction (logits are O(5) here; exp is safe in f32).
    Normalization runs off the critical path: unnormalized OT + denominator
    row are evicted to SBUF, then reciprocal (DVE) + partition_broadcast
    (GpSimd) + in-place multiply (DVE) overlap the next pair's matmuls.
  - proj for batch 0 is emitted between the two batches' attention so its
    PSUM use (borrowed from the ST tag), evictions, and output DMA overlap
    batch 1's attention.

Matmul dtypes: float32r (~1e-4 rel err) for qkv/S/proj; bf16 for the P@V
matmul (P in [0,1]; errors average out over 2048 keys).
"""
import sys

sys.path.insert(0, "/opt/trn_rl_repo")

import numpy as np

B = 2
N = 2048
C = 1024
H = 16
D = 64
R = B * N            # 4096 flattened rows
NCORES = 8
HPC = H // NCORES    # heads per core = 2
SCALE = 1.0 / np.sqrt(D)  # 0.125

_NC_CACHE = None


def build_nc():
    import concourse.bass as bass
    import concourse.tile as tile
    from concourse import bacc, mybir
    from concourse.masks import make_identity

    F32 = mybir.dt.float32
    F32R = mybir.dt.float32r
    BF16 = mybir.dt.float16  # fp16: same PE speed as bf16, 8x the mantissa
    Exp = mybir.ActivationFunctionType.Exp

    nc = bacc.Bacc("TRN2", target_bir_lowering=False, debug=False,
                   num_devices=NCORES)

    xT_d = nc.declare_dram_parameter("xT", [C, R], F32R, isOutput=False)
    wqkvT_d = nc.declare_dram_parameter("wqkvT", [C, 3 * 2 * D], F32R,
                                        isOutput=False)
    wprojT_d = nc.declare_dram_parameter("wprojT", [2 * D, C], F32R,
                                         isOutput=False)
    y_d = nc.declare_dram_parameter("y", [R, C], F32, isOutput=True)

    O3 = 3 * 2 * D   # 384 qkv output rows per core
    CC = C // 128    # 8 contraction chunks
    NMC = N // 128   # 16 key chunks per (b, head)

    with tile.TileContext(nc) as tc:
        with (
            tc.tile_pool(name="const", bufs=1) as const,
            tc.tile_pool(name="qkvT", bufs=1) as qkvp,
            tc.tile_pool(name="vprime", bufs=1) as vpp,
            tc.tile_pool(name="otbuf", bufs=1) as otp,
        ):
            # ---- constants ----
            wqkv_sb = const.tile([128, CC, O3], F32R)
            nc.sync.dma_start(
                wqkv_sb[:], wqkvT_d.rearrange("(a p) o -> p a o", p=128))
            wproj_sb = const.tile([128, C], F32R)
            nc.sync.dma_start(wproj_sb[:], wprojT_d[:])
            ident = const.tile([128, 128], F32)
            make_identity(nc, ident[:])

            # ---- persistent activations ----
            qT = qkvp.tile([128, R], BF16)   # rows: [q_h0 | q_h1] d-major
            kT = qkvp.tile([128, R], BF16)
            vprime = [[vpp.tile([128, NMC, D + 1], BF16, tag=f"vp{b}{hl}",
                                name=f"vp{b}{hl}")
                       for hl in range(HPC)] for b in range(B)]
            ot = otp.tile([128, R], F32R)    # normalized attention out, c-major

            # ================= phase 1: qkv projection =================
            with (
                tc.tile_pool(name="vtbuf", bufs=1) as vtp,
                tc.tile_pool(name="xt", bufs=4) as xtp,
                tc.tile_pool(name="qkps", bufs=2, space="PSUM") as qkps,
                tc.tile_pool(name="vtps", bufs=2, space="PSUM") as vtps,
                nc.named_scope("qkv"),
            ):
                vT = vtp.tile([128, R], F32)
                # ones rows of V' (bf16 1.0 exact); transposes fill [:, :, 0:D]
                for b in range(B):
                    for hl in range(HPC):
                        nc.gpsimd.memset(vprime[b][hl][:, :, D:D + 1], 1.0)

                for rb in range(R // 512):
                    col0 = rb * 512
                    xt = xtp.tile([128, CC, 512], F32R, tag="xt")
                    nc.sync.dma_start(
                        xt[:],
                        xT_d[:, col0:col0 + 512].rearrange(
                            "(a p) r -> p a r", p=128))
                    for ob in range(3):
                        dst = (qT, kT, vT)[ob]
                        ps = qkps.tile([128, 512], F32, tag="qk")
                        for cc in range(CC):
                            nc.tensor.matmul(
                                ps[:],
                                wqkv_sb[:, cc, ob * 128:(ob + 1) * 128],
                                xt[:, cc, :],
                                start=(cc == 0), stop=(cc == CC - 1),
                            )
                        nc.vector.tensor_copy(dst[:, col0:col0 + 512], ps[:])

                    # V' transposes for the v columns that just landed
                    for hl in range(HPC):
                        for i128 in range(4):
                            col = col0 + i128 * 128
                            b = col // N
                            mc = (col % N) // 128
                            pt = vtps.tile([128, D], F32, tag="vt")
                            nc.tensor.transpose(
                                pt[:],
                                vT[hl * D:(hl + 1) * D, col:col + 128],
                                ident[hl * D:(hl + 1) * D,
                                      hl * D:(hl + 1) * D],
                            )
                            nc.vector.tensor_copy(
                                vprime[b][hl][:, mc, 0:D], pt[:])

            # ============ phase 2+3: attention / normalize / proj ==========
            with (
                tc.tile_pool(name="stps", bufs=2, space="PSUM") as stps,
                tc.tile_pool(name="otps", bufs=1, space="PSUM") as otps,
                tc.tile_pool(name="et", bufs=3) as etp,
                tc.tile_pool(name="small", bufs=2) as small,
                tc.tile_pool(name="ysb", bufs=4) as ysbp,
            ):
                def attention_pair(b, hl):
                    p0 = hl * D
                    rlo = b * N
                    otp_ps = otps.tile([D + 1, N], F32, tag="ot", name="otps")
                    for mc in range(NMC):
                        kslice = kT[p0:p0 + D,
                                    rlo + mc * 128:rlo + (mc + 1) * 128]
                        st_tiles = []
                        for half in range(2):
                            st = stps.tile([128, 1024], F32, tag="st",
                                           name="st")
                            for j in range(2):
                                qc = half * 1024 + j * 512
                                nc.tensor.matmul(
                                    st[:, j * 512:(j + 1) * 512],
                                    kslice,
                                    qT[p0:p0 + D, rlo + qc:rlo + qc + 512],
                                    start=True, stop=True,
                                )
                            st_tiles.append(st)
                        et = etp.tile([128, N], BF16, tag="et", name="et")
                        for half in range(2):
                            nc.scalar.activation(
                                et[:, half * 1024:(half + 1) * 1024],
                                st_tiles[half][:], Exp, scale=SCALE)
                        for qc in range(4):
                            nc.tensor.matmul(
                                otp_ps[:, qc * 512:(qc + 1) * 512],
                                vprime[b][hl][:, mc, :],
                                et[:, qc * 512:(qc + 1) * 512],
                                start=(mc == 0), stop=(mc == NMC - 1),
                            )
                    # single fast eviction releases the OT' psum; the
                    # normalize chain below runs off the critical path.
                    otu = small.tile([D + 1, N], F32, tag="otu", name="otu")
                    nc.vector.tensor_copy(otu[:], otp_ps[:])
                    lnd = small.tile([1, N], F32, tag="lnd", name="lnd")
                    nc.scalar.activation(lnd[:], otu[D:D + 1, :],
                                         mybir.ActivationFunctionType.Ln)
                    rinv = small.tile([1, N], F32, tag="rinv", name="rinv")
                    nc.scalar.activation(rinv[:], lnd[:], Exp, scale=-1.0)
                    rbig = small.tile([D, N], F32, tag="rbig", name="rbig")
                    nc.gpsimd.partition_broadcast(rbig[:], rinv[:])
                    nc.vector.tensor_mul(
                        ot[p0:p0 + D, rlo:rlo + N], otu[0:D, :], rbig[:])

                def proj_b(b):
                    for rbi in range(N // 128):
                        rb = b * (N // 128) + rbi
                        yp = stps.tile([128, C], F32, tag="st", name="yp")
                        for j in range(2):
                            nc.tensor.matmul(
                                yp[:, j * 512:(j + 1) * 512],
                                ot[:, rb * 128:(rb + 1) * 128],
                                wproj_sb[:, j * 512:(j + 1) * 512],
                                start=True, stop=True,
                            )
                        ysb = ysbp.tile([128, C], F32, tag="ysb", name="ysb")
                        nc.vector.tensor_copy(ysb[:, 0:512], yp[:, 0:512])
                        nc.scalar.copy(ysb[:, 512:1024], yp[:, 512:1024])
                        nc.sync.dma_start(
                            y_d[rb * 128:(rb + 1) * 128, :], ysb[:])

                with nc.named_scope("attn00"):
                    attention_pair(0, 0)
                with nc.named_scope("attn01"):
                    attention_pair(0, 1)
                with nc.named_scope("attn10"):
                    attention_pair(1, 0)
                with nc.named_scope("proj0"):
                    proj_b(0)
                with nc.named_scope("attn11"):
                    attention_pair(1, 1)
                with nc.named_scope("proj1"):
                    proj_b(1)

    nc.compile()
    return nc


def get_nc():
    global _NC_CACHE
    if _NC_CACHE is None:
        _NC_CACHE = build_nc()
    return _NC_CACHE


def make_in_maps(x, w_qkv, w_proj):
    x = np.asarray(x, dtype=np.float32)
    w_qkv = np.asarray(w_qkv, dtype=np.float32)
    w_proj = np.asarray(w_proj, dtype=np.float32)
    xT = np.ascontiguousarray(x.reshape(R, C).T)
    in_maps = []
    for i in range(NCORES):
        h0, h1 = HPC * i, HPC * i + 1
        rows = []
        for part in range(3):  # q, k, v
            for h in (h0, h1):
                lo = part * C + h * D
                rows.append(w_qkv[lo:lo + D])
        w_slice = np.concatenate(rows, axis=0)           # [384, 1024]
        wqkvT = np.ascontiguousarray(w_slice.T)          # [1024, 384]
        cols = np.r_[h0 * D:(h0 + 1) * D, h1 * D:(h1 + 1) * D]
        wprojT = np.ascontiguousarray(w_proj[:, cols].T)  # [128, 1024]
        in_maps.append({"xT": xT, "wqkvT": wqkvT, "wprojT": wprojT})
    return in_maps


def kernel(x, w_qkv, w_proj, b_proj):
    from concourse.bass_utils import run_bass_kernel_spmd

    nc = get_nc()
    in_maps = make_in_maps(x, w_qkv, w_proj)
    res = run_bass_kernel_spmd(nc, in_maps, core_ids=list(range(NCORES)))
    y = np.zeros((R, C), dtype=np.float32)
    for r in res.results:
        y += r["y"]
    y += np.asarray(b_proj, dtype=np.float32)[None, :]
    return y.reshape(B, N, C)


# revision 10
# speedup vs baseline: 1.1859x; 1.0801x over previous
"""Multi-head attention block (B=2, N=2048, C=1024, H=16, D=64) on 8 TRN2
NeuronCores.

Sharding: tensor-parallel over heads — 2 heads per core, both batch elements.
Each core computes qkv for its 2 heads, full attention for its 4 (batch, head)
pairs, and a partial output projection over its 128 columns of the attention
output. The host sums the 8 partial projections and adds the bias.

Device-side layout (per core):
  - host feeds x transposed (xT [1024, 4096]) plus per-core transposed weight
    slices, so no activation transposes are needed on device for the linears.
  - qkvT [o, r] = wT_slice.T @ xT computed with o on partitions: q/k land
    d-major ([2*64, 4096]) ready to be S-matmul operands; v is PE-transposed
    into m-major V' tiles [128, 65] with an appended ones row, so the P@V
    matmul accumulates the softmax denominator for free.
  - S computed transposed (ST [keys, queries]) so exp(ST) is directly the
    moving operand of the P@V matmul — no P transposes.
  - softmax has no max-subtraction (logits are O(5) here; exp is safe in f32).
    Normalization runs off the critical path: unnormalized OT + denominator
    row are evicted to SBUF, then reciprocal (DVE) + partition_broadcast
    (GpSimd) + in-place multiply (DVE) overlap the next pair's matmuls.
  - proj for batch 0 is emitted between the two batches' attention so its
    PSUM use (borrowed from the ST tag), evictions, and output DMA overlap
    batch 1's attention.

Matmul dtypes: float32r (~1e-4 rel err) for qkv/S/proj; bf16 for the P@V
matmul (P in [0,1]; errors average out over 2048 keys).
"""
import sys

sys.path.insert(0, "/opt/trn_rl_repo")

import numpy as np

B = 2
N = 2048
C = 1024
H = 16
D = 64
R = B * N            # 4096 flattened rows
NCORES = 8
HPC = H // NCORES    # heads per core = 2
SCALE = 1.0 / np.sqrt(D)  # 0.125

_NC_CACHE = None


def build_nc():
    import concourse.bass as bass
    import concourse.tile as tile
    from concourse import bacc, mybir
    from concourse.masks import make_identity

    F32 = mybir.dt.float32
    F32R = mybir.dt.float32r
    BF16 = mybir.dt.float16  # fp16: same PE speed as bf16, 8x the mantissa
    Exp = mybir.ActivationFunctionType.Exp

    nc = bacc.Bacc("TRN2", target_bir_lowering=False, debug=False,
                   num_devices=NCORES)

    xT_d = nc.declare_dram_parameter("xT", [C, R], BF16, isOutput=False)
    wqkvT_d = nc.declare_dram_parameter("wqkvT", [C, 3 * 2 * D], BF16,
                                        isOutput=False)
    wprojT_d = nc.declare_dram_parameter("wprojT", [2 * D, C], BF16,
                                         isOutput=False)
    y_d = nc.declare_dram_parameter("y", [R, C], F32, isOutput=True)

    O3 = 3 * 2 * D   # 384 qkv output rows per core
    CC = C // 128    # 8 contraction chunks
    NMC = N // 128   # 16 key chunks per (b, head)

    with tile.TileContext(nc) as tc:
        with (
            tc.tile_pool(name="const", bufs=1) as const,
            tc.tile_pool(name="qkvT", bufs=1) as qkvp,
            tc.tile_pool(name="vprime", bufs=1) as vpp,
            tc.tile_pool(name="otbuf", bufs=1) as otp,
        ):
            # ---- constants ----
            wqkv_sb = const.tile([128, CC, O3], BF16)
            wq_r = wqkvT_d.rearrange("(a p) o -> p a o", p=128)
            for cc in range(CC):
                nc.sync.dma_start(wqkv_sb[:, cc, :], wq_r[:, cc, :])
            wproj_sb = const.tile([128, C], BF16)
            nc.sync.dma_start(wproj_sb[:], wprojT_d[:])
            ident = const.tile([128, 128], BF16)
            make_identity(nc, ident[:])

            # ---- persistent activations ----
            qT = qkvp.tile([128, R], BF16)   # rows: [q_h0 | q_h1] d-major
            kT = qkvp.tile([128, R], BF16)
            vprime = [[vpp.tile([128, NMC, D + 1], BF16, tag=f"vp{b}{hl}",
                                name=f"vp{b}{hl}")
                       for hl in range(HPC)] for b in range(B)]
            ot = otp.tile([128, R], BF16)    # normalized attention out, c-major

            # ================= phase 1: qkv projection =================
            with (
                tc.tile_pool(name="vtbuf", bufs=1) as vtp,
                tc.tile_pool(name="xt", bufs=4) as xtp,
                tc.tile_pool(name="qkps", bufs=2, space="PSUM") as qkps,
                tc.tile_pool(name="vtps", bufs=2, space="PSUM") as vtps,
                nc.named_scope("qkv"),
            ):
                vT = vtp.tile([128, R], BF16)
                # ones rows of V' (bf16 1.0 exact); transposes fill [:, :, 0:D]
                for b in range(B):
                    for hl in range(HPC):
                        nc.gpsimd.memset(vprime[b][hl][:, :, D:D + 1], 1.0)

                for rb in range(R // 512):
                    col0 = rb * 512
                    xt = xtp.tile([128, CC, 512], BF16, tag="xt")
                    nc.sync.dma_start(
                        xt[:],
                        xT_d[:, col0:col0 + 512].rearrange(
                            "(a p) r -> p a r", p=128))
                    for ob in range(3):
                        dst = (qT, kT, vT)[ob]
                        ps = qkps.tile([128, 512], F32, tag="qk")
                        for cc in range(CC):
                            nc.tensor.matmul(
                                ps[:],
                                wqkv_sb[:, cc, ob * 128:(ob + 1) * 128],
                                xt[:, cc, :],
                                start=(cc == 0), stop=(cc == CC - 1),
                            )
                        nc.vector.tensor_copy(dst[:, col0:col0 + 512], ps[:])

                    # V' transposes for the v columns that just landed
                    for hl in range(HPC):
                        for i128 in range(4):
                            col = col0 + i128 * 128
                            b = col // N
                            mc = (col % N) // 128
                            pt = vtps.tile([128, D], BF16, tag="vt")
                            nc.tensor.transpose(
                                pt[:],
                                vT[hl * D:(hl + 1) * D, col:col + 128],
                                ident[hl * D:(hl + 1) * D,
                                      hl * D:(hl + 1) * D],
                            )
                            nc.vector.tensor_copy(
                                vprime[b][hl][:, mc, 0:D], pt[:])

            # ============ phase 2+3: attention / normalize / proj ==========
            with (
                tc.tile_pool(name="stps", bufs=2, space="PSUM") as stps,
                tc.tile_pool(name="otps", bufs=1, space="PSUM") as otps,
                tc.tile_pool(name="et", bufs=3) as etp,
                tc.tile_pool(name="small", bufs=2) as small,
                tc.tile_pool(name="ysb", bufs=4) as ysbp,
            ):
                def attention_pair(b, hl):
                    p0 = hl * D
                    rlo = b * N
                    otp_ps = otps.tile([D + 1, N], F32, tag="ot", name="otps")
                    for mc in range(NMC):
                        kslice = kT[p0:p0 + D,
                                    rlo + mc * 128:rlo + (mc + 1) * 128]
                        st_tiles = []
                        for half in range(2):
                            st = stps.tile([128, 1024], F32, tag="st",
                                           name="st")
                            for j in range(2):
                                qc = half * 1024 + j * 512
                                nc.tensor.matmul(
                                    st[:, j * 512:(j + 1) * 512],
                                    kslice,
                                    qT[p0:p0 + D, rlo + qc:rlo + qc + 512],
                                    start=True, stop=True,
                                )
                            st_tiles.append(st)
                        et = etp.tile([128, N], BF16, tag="et", name="et")
                        for half in range(2):
                            nc.scalar.activation(
                                et[:, half * 1024:(half + 1) * 1024],
                                st_tiles[half][:], Exp, scale=SCALE)
                        for qc in range(4):
                            nc.tensor.matmul(
                                otp_ps[:, qc * 512:(qc + 1) * 512],
                                vprime[b][hl][:, mc, :],
                                et[:, qc * 512:(qc + 1) * 512],
                                start=(mc == 0), stop=(mc == NMC - 1),
                            )
                    # single fast eviction releases the OT' psum; the
                    # normalize chain below runs off the critical path.
                    otu = small.tile([D + 1, N], F32, tag="otu", name="otu")
                    nc.vector.tensor_copy(otu[:], otp_ps[:])
                    lnd = small.tile([1, N], F32, tag="lnd", name="lnd")
                    nc.scalar.activation(lnd[:], otu[D:D + 1, :],
                                         mybir.ActivationFunctionType.Ln)
                    rinv = small.tile([1, N], F32, tag="rinv", name="rinv")
                    nc.scalar.activation(rinv[:], lnd[:], Exp, scale=-1.0)
                    rbig = small.tile([D, N], F32, tag="rbig", name="rbig")
                    nc.gpsimd.partition_broadcast(rbig[:], rinv[:])
                    nc.vector.tensor_mul(
                        ot[p0:p0 + D, rlo:rlo + N], otu[0:D, :], rbig[:])

                def proj_b(b):
                    for rbi in range(N // 128):
                        rb = b * (N // 128) + rbi
                        yp = stps.tile([128, C], F32, tag="st", name="yp")
                        for j in range(2):
                            nc.tensor.matmul(
                                yp[:, j * 512:(j + 1) * 512],
                                ot[:, rb * 128:(rb + 1) * 128],
                                wproj_sb[:, j * 512:(j + 1) * 512],
                                start=True, stop=True,
                            )
                        ysb = ysbp.tile([128, C], F32, tag="ysb", name="ysb")
                        nc.vector.tensor_copy(ysb[:], yp[:])
                        nc.sync.dma_start(
                            y_d[rb * 128:(rb + 1) * 128, :], ysb[:])

                with nc.named_scope("attn00"):
                    attention_pair(0, 0)
                with nc.named_scope("attn01"):
                    attention_pair(0, 1)
                with nc.named_scope("attn10"):
                    attention_pair(1, 0)
                with nc.named_scope("proj0"):
                    proj_b(0)
                with nc.named_scope("attn11"):
                    attention_pair(1, 1)
                with nc.named_scope("proj1"):
                    proj_b(1)

    nc.compile()
    return nc


def get_nc():
    global _NC_CACHE
    if _NC_CACHE is None:
        _NC_CACHE = build_nc()
    return _NC_CACHE


def make_in_maps(x, w_qkv, w_proj):
    x = np.asarray(x, dtype=np.float32)
    w_qkv = np.asarray(w_qkv, dtype=np.float32)
    w_proj = np.asarray(w_proj, dtype=np.float32)
    xT = np.ascontiguousarray(x.reshape(R, C).T.astype(np.float16))
    in_maps = []
    for i in range(NCORES):
        h0, h1 = HPC * i, HPC * i + 1
        rows = []
        for part in range(3):  # q, k, v
            for h in (h0, h1):
                lo = part * C + h * D
                rows.append(w_qkv[lo:lo + D])
        w_slice = np.concatenate(rows, axis=0)           # [384, 1024]
        wqkvT = np.ascontiguousarray(w_slice.T.astype(np.float16))
        cols = np.r_[h0 * D:(h0 + 1) * D, h1 * D:(h1 + 1) * D]
        wprojT = np.ascontiguousarray(w_proj[:, cols].T.astype(np.float16))
        in_maps.append({"xT": xT, "wqkvT": wqkvT, "wprojT": wprojT})
    return in_maps


def kernel(x, w_qkv, w_proj, b_proj):
    from concourse.bass_utils import run_bass_kernel_spmd

    nc = get_nc()
    in_maps = make_in_maps(x, w_qkv, w_proj)
    res = run_bass_kernel_spmd(nc, in_maps, core_ids=list(range(NCORES)))
    y = np.zeros((R, C), dtype=np.float32)
    for r in res.results:
        y += r["y"]
    y += np.asarray(b_proj, dtype=np.float32)[None, :]
    return y.reshape(B, N, C)


# revision 11
# speedup vs baseline: 1.2246x; 1.0326x over previous
"""Multi-head attention block (B=2, N=2048, C=1024, H=16, D=64) on 8 TRN2
NeuronCores.

Sharding: tensor-parallel over heads — 2 heads per core, both batch elements.
Each core computes qkv for its 2 heads, full attention for its 4 (batch, head)
pairs, and a partial output projection over its 128 columns of the attention
output. The host sums the 8 partial projections and adds the bias.

Device-side layout (per core):
  - host feeds x transposed (xT [1024, 4096]) plus per-core transposed weight
    slices, so no activation transposes are needed on device for the linears.
  - qkvT [o, r] = wT_slice.T @ xT computed with o on partitions: q/k land
    d-major ([2*64, 4096]) ready to be S-matmul operands; v is PE-transposed
    into m-major V' tiles [128, 65] with an appended ones row, so the P@V
    matmul accumulates the softmax denominator for free.
  - S computed transposed (ST [keys, queries]) so exp(ST) is directly the
    moving operand of the P@V matmul — no P transposes.
  - softmax has no max-subtraction (logits are O(5) here; exp is safe in f32).
    Normalization runs off the critical path: unnormalized OT + denominator
    row are evicted to SBUF, then reciprocal (DVE) + partition_broadcast
    (GpSimd) + in-place multiply (DVE) overlap the next pair's matmuls.
  - proj for batch 0 is emitted between the two batches' attention so its
    PSUM use (borrowed from the ST tag), evictions, and output DMA overlap
    batch 1's attention.

Matmul dtypes: float32r (~1e-4 rel err) for qkv/S/proj; bf16 for the P@V
matmul (P in [0,1]; errors average out over 2048 keys).
"""
import sys

sys.path.insert(0, "/opt/trn_rl_repo")

import numpy as np

B = 2
N = 2048
C = 1024
H = 16
D = 64
R = B * N            # 4096 flattened rows
NCORES = 8
HPC = H // NCORES    # heads per core = 2
SCALE = 1.0 / np.sqrt(D)  # 0.125

_NC_CACHE = None


def build_nc():
    import concourse.bass as bass
    import concourse.tile as tile
    from concourse import bacc, mybir
    from concourse.masks import make_identity

    F32 = mybir.dt.float32
    F32R = mybir.dt.float32r
    BF16 = mybir.dt.float16  # fp16: same PE speed as bf16, 8x the mantissa
    Exp = mybir.ActivationFunctionType.Exp

    nc = bacc.Bacc("TRN2", target_bir_lowering=False, debug=False,
                   num_devices=NCORES)

    xT_d = nc.declare_dram_parameter("xT", [C, R], BF16, isOutput=False)
    wqkvT_d = nc.declare_dram_parameter("wqkvT", [C, 3 * 2 * D], BF16,
                                        isOutput=False)
    wprojT_d = nc.declare_dram_parameter("wprojT", [2 * D, C], BF16,
                                         isOutput=False)
    y_d = nc.declare_dram_parameter("y", [R, C], F32, isOutput=True)

    O3 = 3 * 2 * D   # 384 qkv output rows per core
    CC = C // 128    # 8 contraction chunks
    NMC = N // 128   # 16 key chunks per (b, head)

    with tile.TileContext(nc) as tc:
        with (
            tc.tile_pool(name="const", bufs=1) as const,
            tc.tile_pool(name="qkvT", bufs=1) as qkvp,
            tc.tile_pool(name="vprime", bufs=1) as vpp,
            tc.tile_pool(name="otbuf", bufs=1) as otp,
        ):
            # ---- constants ----
            wqkv_sb = const.tile([128, CC, O3], BF16)
            wq_r = wqkvT_d.rearrange("(a p) o -> p a o", p=128)
            for cc in range(CC):
                nc.sync.dma_start(wqkv_sb[:, cc, :], wq_r[:, cc, :])
            wproj_sb = const.tile([128, C], BF16)
            nc.sync.dma_start(wproj_sb[:], wprojT_d[:])
            ident = const.tile([128, 128], BF16)
            make_identity(nc, ident[:])

            # ---- persistent activations ----
            qT = qkvp.tile([128, R], BF16)   # rows: [q_h0 | q_h1] d-major
            kT = qkvp.tile([128, R], BF16)
            vprime = [[vpp.tile([128, NMC, D + 1], BF16, tag=f"vp{b}{hl}",
                                name=f"vp{b}{hl}")
                       for hl in range(HPC)] for b in range(B)]
            ot = otp.tile([128, R], BF16)    # normalized attention out, c-major

            # ================= phase 1: qkv projection =================
            with (
                tc.tile_pool(name="vtbuf", bufs=1) as vtp,
                tc.tile_pool(name="xt", bufs=4) as xtp,
                tc.tile_pool(name="qkps", bufs=2, space="PSUM") as qkps,
                tc.tile_pool(name="vtps", bufs=2, space="PSUM") as vtps,
                nc.named_scope("qkv"),
            ):
                vT = vtp.tile([128, R], BF16)
                # ones rows of V' (bf16 1.0 exact); transposes fill [:, :, 0:D]
                for b in range(B):
                    for hl in range(HPC):
                        nc.gpsimd.memset(vprime[b][hl][:, :, D:D + 1], 1.0)

                for rb in range(R // 512):
                    col0 = rb * 512
                    xt = xtp.tile([128, CC, 512], BF16, tag="xt")
                    nc.sync.dma_start(
                        xt[:],
                        xT_d[:, col0:col0 + 512].rearrange(
                            "(a p) r -> p a r", p=128))
                    for ob in range(3):
                        dst = (qT, kT, vT)[ob]
                        ps = qkps.tile([128, 512], F32, tag="qk")
                        for cc in range(CC):
                            nc.tensor.matmul(
                                ps[:],
                                wqkv_sb[:, cc, ob * 128:(ob + 1) * 128],
                                xt[:, cc, :],
                                start=(cc == 0), stop=(cc == CC - 1),
                            )
                        nc.vector.tensor_copy(dst[:, col0:col0 + 512], ps[:])

                    # V' transposes for the v columns that just landed
                    for hl in range(HPC):
                        for i128 in range(4):
                            col = col0 + i128 * 128
                            b = col // N
                            mc = (col % N) // 128
                            pt = vtps.tile([128, D], BF16, tag="vt")
                            nc.tensor.transpose(
                                pt[:],
                                vT[hl * D:(hl + 1) * D, col:col + 128],
                                ident[hl * D:(hl + 1) * D,
                                      hl * D:(hl + 1) * D],
                            )
                            nc.vector.tensor_copy(
                                vprime[b][hl][:, mc, 0:D], pt[:])

            # ============ phase 2+3: attention / normalize / proj ==========
            with (
                tc.tile_pool(name="stps", bufs=3, space="PSUM") as stps,
                tc.tile_pool(name="otps", bufs=1, space="PSUM") as otps,
                tc.tile_pool(name="et", bufs=4) as etp,
                tc.tile_pool(name="small", bufs=3) as small,
                tc.tile_pool(name="ysb", bufs=4) as ysbp,
            ):
                def attention_pair(b, hl):
                    p0 = hl * D
                    rlo = b * N
                    # two q-half passes: OT' fits 2 banks, leaving room for
                    # 3 ST buffers so the PE can run ahead of the exp stream
                    # (keeps the HAM clock-gate at 8/8).
                    for qh in range(2):
                        q0 = rlo + qh * 1024
                        otp_ps = otps.tile([D + 1, 1024], F32, tag="ot",
                                           name="otps")
                        for mc in range(NMC):
                            kslice = kT[p0:p0 + D,
                                        rlo + mc * 128:rlo + (mc + 1) * 128]
                            st = stps.tile([128, 1024], F32, tag="st",
                                           name="st")
                            for j in range(2):
                                nc.tensor.matmul(
                                    st[:, j * 512:(j + 1) * 512],
                                    kslice,
                                    qT[p0:p0 + D,
                                       q0 + j * 512:q0 + (j + 1) * 512],
                                    start=True, stop=True,
                                )
                            et = etp.tile([128, 1024], BF16, tag="et",
                                          name="et")
                            nc.scalar.activation(et[:], st[:], Exp,
                                                 scale=SCALE)
                            for j in range(2):
                                nc.tensor.matmul(
                                    otp_ps[:, j * 512:(j + 1) * 512],
                                    vprime[b][hl][:, mc, :],
                                    et[:, j * 512:(j + 1) * 512],
                                    start=(mc == 0), stop=(mc == NMC - 1),
                                )
                        # fast eviction releases the OT' psum; the normalize
                        # chain below runs off the critical path on DVE/GpSimd.
                        otu = small.tile([D + 1, 1024], F32, tag="otu",
                                         name="otu")
                        nc.vector.tensor_copy(otu[:], otp_ps[:])
                        rinv = small.tile([1, 1024], F32, tag="rinv",
                                          name="rinv")
                        nc.vector.reciprocal(rinv[:], otu[D:D + 1, :])
                        rbig = small.tile([D, 1024], F32, tag="rbig",
                                          name="rbig")
                        nc.gpsimd.partition_broadcast(rbig[:], rinv[:])
                        nc.vector.tensor_mul(
                            ot[p0:p0 + D, q0:q0 + 1024], otu[0:D, :],
                            rbig[:])

                def proj_b(b):
                    for rbi in range(N // 128):
                        rb = b * (N // 128) + rbi
                        yp = stps.tile([128, C], F32, tag="st", name="yp")
                        for j in range(2):
                            nc.tensor.matmul(
                                yp[:, j * 512:(j + 1) * 512],
                                ot[:, rb * 128:(rb + 1) * 128],
                                wproj_sb[:, j * 512:(j + 1) * 512],
                                start=True, stop=True,
                            )
                        ysb = ysbp.tile([128, C], F32, tag="ysb", name="ysb")
                        nc.vector.tensor_copy(ysb[:], yp[:])
                        nc.sync.dma_start(
                            y_d[rb * 128:(rb + 1) * 128, :], ysb[:])

                with nc.named_scope("attn00"):
                    attention_pair(0, 0)
                with nc.named_scope("attn01"):
                    attention_pair(0, 1)
                with nc.named_scope("attn10"):
                    attention_pair(1, 0)
                with nc.named_scope("proj0"):
                    proj_b(0)
                with nc.named_scope("attn11"):
                    attention_pair(1, 1)
                with nc.named_scope("proj1"):
                    proj_b(1)

    nc.compile()
    return nc


def get_nc():
    global _NC_CACHE
    if _NC_CACHE is None:
        _NC_CACHE = build_nc()
    return _NC_CACHE


def make_in_maps(x, w_qkv, w_proj):
    x = np.asarray(x, dtype=np.float32)
    w_qkv = np.asarray(w_qkv, dtype=np.float32)
    w_proj = np.asarray(w_proj, dtype=np.float32)
    xT = np.ascontiguousarray(x.reshape(R, C).T.astype(np.float16))
    in_maps = []
    for i in range(NCORES):
        h0, h1 = HPC * i, HPC * i + 1
        rows = []
        for part in range(3):  # q, k, v
            for h in (h0, h1):
                lo = part * C + h * D
                rows.append(w_qkv[lo:lo + D])
        w_slice = np.concatenate(rows, axis=0)           # [384, 1024]
        wqkvT = np.ascontiguousarray(w_slice.T.astype(np.float16))
        cols = np.r_[h0 * D:(h0 + 1) * D, h1 * D:(h1 + 1) * D]
        wprojT = np.ascontiguousarray(w_proj[:, cols].T.astype(np.float16))
        in_maps.append({"xT": xT, "wqkvT": wqkvT, "wprojT": wprojT})
    return in_maps


def kernel(x, w_qkv, w_proj, b_proj):
    from concourse.bass_utils import run_bass_kernel_spmd

    nc = get_nc()
    in_maps = make_in_maps(x, w_qkv, w_proj)
    res = run_bass_kernel_spmd(nc, in_maps, core_ids=list(range(NCORES)))
    y = np.zeros((R, C), dtype=np.float32)
    for r in res.results:
        y += r["y"]
    y += np.asarray(b_proj, dtype=np.float32)[None, :]
    return y.reshape(B, N, C)
